# revision 8
# baseline (speedup 1.0000x reference)
"""Trainium2 Bass kernel for nn_MLoss_68066641707785 (topk_masking loss).

Computes, for x, y of shape [128, 43264, 5] (fp32):
    m        = (y[:,:,0] > 0.5)
    face_num = sum(m)
    scale    = 1 + 1/face_num
    diff_box = scale * sum(m * (x[:,:,1:5]-y[:,:,1:5])^2) / (face_num*4)
    bce      = -(t*log(p) + (1-t)*log(1-p)),  p = x[:,:,0], t = y[:,:,0]
    diff_c   = scale * sum(m * bce) / face_num
    diff_bg  = 0.5 * mean(-log(1-p))
    out      = diff_box + diff_c + diff_bg          (scalar fp32)

Strategy: pure data-parallel over the batch axis (16 batches per core x 8
cores), bf16 inputs (tolerance 2e-2 leaves orders of magnitude of slack).

v3 schedule (trace-driven):
  - DVE is the critical engine (~40us busy).  Small first tile so DVE
    starts ~10.5us (right after the fixed ~7us framework preamble + the
    first small conf DMA); it then runs gap-free.
  - Tile sizes ramp up then decay [512,1280,1344,1344,512,256,160]:
    big middle tiles keep bb DMA rows >=20KB (full ~390 B/ns DMA rate,
    v2's uniform small tiles dropped it to 323), decaying tail tiles so
    the last Squares (which can only start after their dm, i.e. near
    DVE's end) are tiny.
  - Two psum accumulation groups: tiles 0..4 close early so their 4
    [1,512] psum->SBUF copies run on ACT while DVE still computes;
    tiles 5-6 accumulate into a second psum row set whose tiny [1,256]
    copies run on the (by then idle) DVE right at the end, in parallel
    with ACT's last Squares.  One merged q output DMA.
  - DVE per-tile order m,u,v,p12,d4,dm; ACT order lp,lq,(prev Square).
Engines: DVE 6.25 cyc/cell (m TS@4x; u,v,p12,d4,dm TT@2x) ~41us busy,
ACT 6 cyc/cell (ln,ln,Square+accum) ~36us, TensorE ones-matmul column
sums of m/p12/lq, DMA 13.85MB.
Host sums strips/psum rows in float64 and applies the final scalars.
"""

import numpy as np

try:
    import ml_dtypes
    from concourse import bacc, bass, mybir, tile
    from concourse.bass_utils import run_bass_kernel_spmd
except ImportError:  # repo not on sys.path in a fresh grading dir
    import sys

    for _p in ("/opt/trn_rl_repo", "/root/.axon_site/_ro/trn_rl_repo"):
        if _p not in sys.path:
            sys.path.insert(0, _p)
    import ml_dtypes
    from concourse import bacc, bass, mybir, tile
    from concourse.bass_utils import run_bass_kernel_spmd

THRESH = 0.5
ALPHA = 0.5

B, N, C = 128, 43264, 5
M = 8                      # cores
BS = B // M                # 16 batches per core
P = 128                    # SBUF partitions
CELLS = BS * N // P        # 5408 cells per partition per core
SIZES = [512, 1280, 1344, 1344, 512, 256, 160]
assert sum(SIZES) == CELLS
T = len(SIZES)
TB = 5                     # tiles [0, TB) -> psum group A, [TB, T) -> group B
OFFS = [sum(SIZES[:j]) for j in range(T)]
QW = 512                   # psum row width for group A (one bank)
QWB = 256                  # psum row width for group B (max late-tile chunk)


def _chunks(ft, qw):
    out, off = [], 0
    while off < ft:
        out.append((off, min(qw, ft - off)))
        off += qw
    return out


_CACHE = {}


def _build():
    f32 = mybir.dt.float32
    bf16 = mybir.dt.bfloat16
    AF = mybir.ActivationFunctionType
    OP = mybir.AluOpType

    nc = bacc.Bacc("TRN2", target_bir_lowering=False, debug=False, num_devices=M)
    cf_d = nc.declare_dram_parameter("cf", [P, 2 * CELLS], bf16, isOutput=False)
    bb_d = nc.declare_dram_parameter("bb", [P, 8 * CELLS], bf16, isOutput=False)
    o_d = nc.declare_dram_parameter("o", [P, T], f32, isOutput=True)
    q_d = nc.declare_dram_parameter("q", [1, 4 * QW + 4 * QWB], f32,
                                    isOutput=True)

    nmm_a = sum(len(_chunks(ft, QW)) for ft in SIZES[:TB])
    nmm_b = sum(len(_chunks(ft, QWB)) for ft in SIZES[TB:])

    with tile.TileContext(nc) as tc:
        with tc.tile_pool(name="io", bufs=3) as io, \
             tc.tile_pool(name="mid", bufs=2) as mid, \
             tc.tile_pool(name="acc", bufs=1) as accp, \
             tc.tile_pool(name="ps", bufs=1, space="PSUM") as ps:
            strips = accp.tile([P, T], f32)   # se per tile
            onesv = accp.tile([P, 1], bf16)
            nc.gpsimd.memset(onesv[:], 1.0)
            qs = accp.tile([1, 4 * QW + 4 * QWB], f32)
            pqa = [ps.tile([1, QW], f32, name=f"pqa{k}")
                   for k in range(4)]               # face,s1,s2,bg A
            pqb = [ps.tile([1, QWB], f32, name=f"pqb{k}")
                   for k in range(4)]               # face,s1,s2,bg B

            imm_a = imm_b = 0
            pend_sq = None
            for j, (ft, off) in enumerate(zip(SIZES, OFFS)):
                cf_t = io.tile([P, 2 * ft], bf16, tag="cf")
                nc.scalar.dma_start(out=cf_t[:],
                                    in_=cf_d[:, 2 * off:2 * (off + ft)])
                bb_t = io.tile([P, 8 * ft], bf16, tag="bb")
                nc.sync.dma_start(out=bb_t[:],
                                  in_=bb_d[:, 8 * off:8 * (off + ft)])
                t_t = cf_t[:, :ft]
                p_t = cf_t[:, ft:]

                # ---- confidence channel ----
                # lp | lq packed adjacent so p12 = uv * lpq is one DVE op
                lpq = mid.tile([P, 2 * ft], bf16, tag="lpq")
                nc.scalar.activation(lpq[:, :ft], p_t, AF.Ln)
                nc.scalar.activation(lpq[:, ft:], p_t, AF.Ln, bias=1.0,
                                     scale=-1.0)
                m = mid.tile([P, ft], bf16, tag="m")
                nc.vector.tensor_scalar(m[:], t_t, THRESH, None, OP.is_gt)
                uv = mid.tile([P, 2 * ft], bf16, tag="uv")
                nc.vector.tensor_tensor(uv[:, :ft], m[:], t_t, OP.mult)
                nc.vector.tensor_tensor(uv[:, ft:], m[:], uv[:, :ft],
                                        OP.subtract)
                p12 = mid.tile([P, 2 * ft], bf16, tag="p12")
                nc.vector.tensor_tensor(p12[:], uv[:], lpq[:], OP.mult)

                # ---- TensorE column-sum accumulation (face, s1, s2, bg) ----
                if j < TB:
                    pq, qw, nmm = pqa, QW, nmm_a
                else:
                    pq, qw, nmm = pqb, QWB, nmm_b
                for (coff, w) in _chunks(ft, qw):
                    imm = imm_a if j < TB else imm_b
                    first, last = imm == 0, imm == nmm - 1
                    srcs = (m[:, coff:coff + w],
                            p12[:, coff:coff + w],
                            p12[:, ft + coff:ft + coff + w],
                            lpq[:, ft + coff:ft + coff + w])
                    for pr, src in zip(pq, srcs):
                        nc.tensor.matmul(pr[:, :w], onesv[:], src,
                                         start=first, stop=last,
                                         skip_group_check=True)
                    if j < TB:
                        imm_a += 1
                    else:
                        imm_b += 1

                if j == TB:
                    # group A closed at tile TB-1's last matmul: its big
                    # [1,512] psum copies run on ACT now, overlapped with
                    # the remaining DVE work
                    for k in range(4):
                        nc.scalar.activation(qs[:, k * QW:(k + 1) * QW],
                                             pqa[k][:], AF.Copy)

                if pend_sq is not None:
                    pdm, pj = pend_sq
                    nc.scalar.activation(pdm[:], pdm[:], AF.Square,
                                         accum_out=strips[:, pj:pj + 1])
                    pend_sq = None

                # ---- box channels (planar segments: ch c at [c*ft,(c+1)*ft)) ----
                d4 = mid.tile([P, 4 * ft], bf16, tag="d4")
                nc.vector.tensor_tensor(d4[:], bb_t[:, :4 * ft],
                                        bb_t[:, 4 * ft:], OP.subtract)
                dm = mid.tile([P, 4 * ft], bf16, tag="dm", bufs=3)
                d4_r = d4[:].rearrange("p (c f) -> p c f", c=4)
                dm_r = dm[:].rearrange("p (c f) -> p c f", c=4)
                m_r = m[:].rearrange("p (c f) -> p c f", c=1)
                m_bc, _ = bass.broadcast_tensor_aps(m_r, d4_r)
                nc.vector.tensor_tensor(dm_r, d4_r, m_bc, OP.mult)
                if j < T - 1:
                    pend_sq = (dm, j)
                else:
                    # group B's tiny [1,256] psum copies on the now-idle
                    # DVE, in parallel with ACT's last Squares
                    for k in range(4):
                        nc.vector.tensor_copy(
                            qs[:, 4 * QW + k * QWB:4 * QW + (k + 1) * QWB],
                            pqb[k][:])
                    nc.scalar.activation(dm[:], dm[:], AF.Square,
                                         accum_out=strips[:, j:j + 1])
            nc.sync.dma_start(out=q_d[:], in_=qs[:])
            nc.sync.dma_start(out=o_d[:], in_=strips[:])

    nc.compile()
    return nc


def _get_nc():
    if "nc" not in _CACHE:
        _CACHE["nc"] = _build()
    return _CACHE["nc"]


def _pack_core(x_sl, y_sl):
    """x_sl, y_sl: [BS, N, 5] fp32 -> bf16 planes for one core."""
    bf = ml_dtypes.bfloat16
    conf = {}
    boxs = {}
    for name, a in (("x", x_sl), ("y", y_sl)):
        conf[name] = np.ascontiguousarray(a[:, :, 0]).reshape(P, CELLS)
        box = a[:, :, 1:5].reshape(P, CELLS, 4)
        boxs[name] = [np.ascontiguousarray(box[:, off:off + ft]
                                           .transpose(0, 2, 1))
                      .reshape(P, 4 * ft) for ft, off in zip(SIZES, OFFS)]
    # cf per tile: [t_seg | p_seg]
    cf = np.concatenate(
        [np.concatenate([conf["y"][:, off:off + ft],
                         conf["x"][:, off:off + ft]], axis=1)
         for ft, off in zip(SIZES, OFFS)], axis=1).astype(bf)
    bb = np.concatenate([np.concatenate([xs, ys], axis=1)
                         for xs, ys in zip(boxs["x"], boxs["y"])],
                        axis=1).astype(bf)
    return {"cf": cf, "bb": bb}


def _in_maps(x, y):
    x = np.asarray(x, dtype=np.float32)
    y = np.asarray(y, dtype=np.float32)
    maps = []
    for i in range(M):
        sl = slice(i * BS, (i + 1) * BS)
        maps.append(_pack_core(x[sl], y[sl]))
    return maps


def _combine(outs):
    """outs: list of M (o [P, T], q [1, 4*QW+4*QWB]) -> scalar fp32 loss."""
    bg = s1 = s2 = se = face = 0.0
    A = 4 * QW
    for o, q in outs:
        o = o.astype(np.float64)
        q = q.astype(np.float64)
        se += o.sum()
        face += q[0, 0:QW].sum() + q[0, A:A + QWB].sum()
        s1 += q[0, QW:2 * QW].sum() + q[0, A + QWB:A + 2 * QWB].sum()
        s2 += q[0, 2 * QW:3 * QW].sum() + q[0, A + 2 * QWB:A + 3 * QWB].sum()
        bg += q[0, 3 * QW:4 * QW].sum() + q[0, A + 3 * QWB:A + 4 * QWB].sum()
    scale = 1.0 + 1.0 / face
    diff_box = scale * se / (face * 4.0)
    diff_c = scale * (-(s1 + s2)) / face
    diff_bg = ALPHA * (-bg) / (B * N)
    return np.asarray(diff_box + diff_c + diff_bg, dtype=np.float32)


def kernel(x, y, **run_kwargs):
    nc = _get_nc()
    res = run_bass_kernel_spmd(nc, _in_maps(x, y), core_ids=list(range(M)),
                               **run_kwargs)
    out = _combine([(res.results[i]["o"], res.results[i]["q"])
                    for i in range(M)])
    if run_kwargs:
        return out, res
    return out


# revision 11
# speedup vs baseline: 1.3508x; 1.3508x over previous
"""Trainium2 Bass kernel for nn_MLoss_68066641707785 (topk_masking loss).

Computes, for x, y of shape [128, 43264, 5] (fp32):
    m        = (y[:,:,0] > 0.5)
    face_num = sum(m)
    scale    = 1 + 1/face_num
    diff_box = scale * sum(m * (x[:,:,1:5]-y[:,:,1:5])^2) / (face_num*4)
    bce      = -(t*log(p) + (1-t)*log(1-p)),  p = x[:,:,0], t = y[:,:,0]
    diff_c   = scale * sum(m * bce) / face_num
    diff_bg  = 0.5 * mean(-log(1-p))
    out      = diff_box + diff_c + diff_bg          (scalar fp32)

Strategy: pure data-parallel over the batch axis (16 batches per core x 8
cores), bf16 inputs (tolerance 2e-2 leaves orders of magnitude of slack).

v3 schedule (trace-driven):
  - DVE is the critical engine (~40us busy).  Small first tile so DVE
    starts ~10.5us (right after the fixed ~7us framework preamble + the
    first small conf DMA); it then runs gap-free.
  - Tile sizes ramp up then decay [512,1280,1344,1344,512,256,160]:
    big middle tiles keep bb DMA rows >=20KB (full ~390 B/ns DMA rate,
    v2's uniform small tiles dropped it to 323), decaying tail tiles so
    the last Squares (which can only start after their dm, i.e. near
    DVE's end) are tiny.
  - Two psum accumulation groups: tiles 0..4 close early so their 4
    [1,512] psum->SBUF copies run on ACT while DVE still computes;
    tiles 5-6 accumulate into a second psum row set whose tiny [1,256]
    copies run on the (by then idle) DVE right at the end, in parallel
    with ACT's last Squares.  One merged q output DMA.
  - DVE per-tile order m,u,v,p12,d4,dm; ACT order lp,lq,(prev Square).
Engines: DVE 6.25 cyc/cell (m TS@4x; u,v,p12,d4,dm TT@2x) ~41us busy,
ACT 6 cyc/cell (ln,ln,Square+accum) ~36us, TensorE ones-matmul column
sums of m/p12/lq, DMA 13.85MB.
Host sums strips/psum rows in float64 and applies the final scalars.
"""

import numpy as np

try:
    import ml_dtypes
    from concourse import bacc, bass, mybir, tile
    from concourse.bass_utils import run_bass_kernel_spmd
except ImportError:  # repo not on sys.path in a fresh grading dir
    import sys

    for _p in ("/opt/trn_rl_repo", "/root/.axon_site/_ro/trn_rl_repo"):
        if _p not in sys.path:
            sys.path.insert(0, _p)
    import ml_dtypes
    from concourse import bacc, bass, mybir, tile
    from concourse.bass_utils import run_bass_kernel_spmd

THRESH = 0.5
ALPHA = 0.5

B, N, C = 128, 43264, 5
M = 8                      # cores
BS = B // M                # 16 batches per core
P = 128                    # SBUF partitions
CELLS = BS * N // P        # 5408 cells per partition per core
SIZES = [512, 1280, 1344, 1344, 512, 256, 160]
assert sum(SIZES) == CELLS
T = len(SIZES)
TB = 5                     # tiles [0, TB) -> psum group A, [TB, T) -> group B
OFFS = [sum(SIZES[:j]) for j in range(T)]
QW = 512                   # psum row width for group A (one bank)
QWB = 256                  # psum row width for group B (max late-tile chunk)


def _chunks(ft, qw):
    out, off = [], 0
    while off < ft:
        out.append((off, min(qw, ft - off)))
        off += qw
    return out


_CACHE = {}


def _build():
    f32 = mybir.dt.float32
    bf16 = mybir.dt.bfloat16
    AF = mybir.ActivationFunctionType
    OP = mybir.AluOpType

    nc = bacc.Bacc("TRN2", target_bir_lowering=False, debug=False, num_devices=M)
    cf_d = nc.declare_dram_parameter("cf", [P, 2 * CELLS], bf16, isOutput=False)
    bb_d = nc.declare_dram_parameter("bb", [P, 8 * CELLS], bf16, isOutput=False)
    o_d = nc.declare_dram_parameter("o", [P, T], f32, isOutput=True)
    q_d = nc.declare_dram_parameter("q", [1, 3 * QW + 3 * QWB], f32,
                                    isOutput=True)

    nmm_a = sum(len(_chunks(ft, QW)) for ft in SIZES[:TB])
    nmm_b = sum(len(_chunks(ft, QWB)) for ft in SIZES[TB:])

    with tile.TileContext(nc) as tc:
        with tc.tile_pool(name="io", bufs=3) as io, \
             tc.tile_pool(name="mid", bufs=2) as mid, \
             tc.tile_pool(name="acc", bufs=1) as accp, \
             tc.tile_pool(name="ps", bufs=1, space="PSUM") as ps:
            strips = accp.tile([P, T], f32)   # se per tile
            onesv = accp.tile([P, 1], bf16)
            nc.gpsimd.memset(onesv[:], 1.0)
            qs = accp.tile([1, 3 * QW + 3 * QWB], f32)
            pqa = [ps.tile([1, QW], f32, name=f"pqa{k}")
                   for k in range(3)]          # face, s12, bg  (group A)
            pqb = [ps.tile([1, QWB], f32, name=f"pqb{k}")
                   for k in range(3)]          # face, s12, bg  (group B)

            imm_a = imm_b = 0
            pend_sq = None
            for j, (ft, off) in enumerate(zip(SIZES, OFFS)):
                cf_t = io.tile([P, 2 * ft], bf16, tag="cf")
                nc.sync.dma_start(out=cf_t[:],
                                  in_=cf_d[:, 2 * off:2 * (off + ft)])
                bb_t = io.tile([P, 8 * ft], bf16, tag="bb")
                nc.sync.dma_start(out=bb_t[:],
                                  in_=bb_d[:, 8 * off:8 * (off + ft)])
                t_t = cf_t[:, :ft]
                p_t = cf_t[:, ft:]

                # ---- confidence channel ----
                # lp | lq packed adjacent so p12 = uv * lpq is one DVE op
                lpq = mid.tile([P, 2 * ft], bf16, tag="lpq")
                nc.scalar.activation(lpq[:, :ft], p_t, AF.Ln)
                nc.scalar.activation(lpq[:, ft:], p_t, AF.Ln, bias=1.0,
                                     scale=-1.0)
                m = mid.tile([P, ft], bf16, tag="m")
                nc.vector.tensor_scalar(m[:], t_t, THRESH, None, OP.is_gt)
                uv = mid.tile([P, 2 * ft], bf16, tag="uv")
                nc.vector.tensor_tensor(uv[:, :ft], m[:], t_t, OP.mult)
                nc.vector.tensor_tensor(uv[:, ft:], m[:], uv[:, :ft],
                                        OP.subtract)
                p12 = mid.tile([P, 2 * ft], bf16, tag="p12")
                nc.vector.tensor_tensor(p12[:], uv[:], lpq[:], OP.mult)

                # ---- TensorE column-sum accumulation (face, s1, s2, bg) ----
                if j < TB:
                    pq, qw, nmm = pqa, QW, nmm_a
                else:
                    pq, qw, nmm = pqb, QWB, nmm_b
                for (coff, w) in _chunks(ft, qw):
                    imm = imm_a if j < TB else imm_b
                    first, last = imm == 0, imm == nmm - 1
                    # (psum col range, src, start, stop): s12 gets both p12
                    # halves accumulated into the same psum row
                    plans = ((pq[0], m[:, coff:coff + w], first, last),
                             (pq[1], p12[:, coff:coff + w], first, False),
                             (pq[1], p12[:, ft + coff:ft + coff + w], False,
                              last),
                             (pq[2], lpq[:, ft + coff:ft + coff + w],
                              first, last))
                    for (pr, src, st, sp) in plans:
                        nc.tensor.matmul(pr[:, :w], onesv[:], src,
                                         start=st, stop=sp,
                                         skip_group_check=True)
                    if j < TB:
                        imm_a += 1
                    else:
                        imm_b += 1

                if j == T - 1:
                    # group A closed at tile TB-1's last matmul: one big
                    # [1,3*QW] psum copy on ACT, after this tile's lns (so
                    # DVE's last p12 isn't gated), overlapped with the
                    # remaining DVE/ACT work
                    for k in range(3):
                        nc.scalar.activation(qs[:, k * QW:(k + 1) * QW],
                                             pqa[k][:], AF.Copy)

                if pend_sq is not None:
                    pdm, pj = pend_sq
                    nc.scalar.activation(pdm[:], pdm[:], AF.Square,
                                         accum_out=strips[:, pj:pj + 1])
                    pend_sq = None

                # ---- box channels (planar segments: ch c at [c*ft,(c+1)*ft)) ----
                d4 = mid.tile([P, 4 * ft], bf16, tag="d4")
                nc.vector.tensor_tensor(d4[:], bb_t[:, :4 * ft],
                                        bb_t[:, 4 * ft:], OP.subtract)
                dm = mid.tile([P, 4 * ft], bf16, tag="dm", bufs=3)
                d4_r = d4[:].rearrange("p (c f) -> p c f", c=4)
                dm_r = dm[:].rearrange("p (c f) -> p c f", c=4)
                m_r = m[:].rearrange("p (c f) -> p c f", c=1)
                m_bc, _ = bass.broadcast_tensor_aps(m_r, d4_r)
                nc.vector.tensor_tensor(dm_r, d4_r, m_bc, OP.mult)
                if j < T - 1:
                    pend_sq = (dm, j)
                else:
                    # group B's single tiny psum copy on the now-idle
                    # DVE, in parallel with ACT's last Squares
                    for k in range(3):
                        nc.vector.tensor_copy(
                            qs[:, 3 * QW + k * QWB:3 * QW + (k + 1) * QWB],
                            pqb[k][:])
                    nc.scalar.activation(dm[:], dm[:], AF.Square,
                                         accum_out=strips[:, j:j + 1])
            nc.sync.dma_start(out=q_d[:], in_=qs[:])
            nc.sync.dma_start(out=o_d[:], in_=strips[:])

    nc.compile()
    return nc


def _get_nc():
    if "nc" not in _CACHE:
        _CACHE["nc"] = _build()
    return _CACHE["nc"]


def _pack_core(x_sl, y_sl):
    """x_sl, y_sl: [BS, N, 5] fp32 -> bf16 planes for one core."""
    bf = ml_dtypes.bfloat16
    conf = {}
    boxs = {}
    for name, a in (("x", x_sl), ("y", y_sl)):
        conf[name] = np.ascontiguousarray(a[:, :, 0]).reshape(P, CELLS)
        box = a[:, :, 1:5].reshape(P, CELLS, 4)
        boxs[name] = [np.ascontiguousarray(box[:, off:off + ft]
                                           .transpose(0, 2, 1))
                      .reshape(P, 4 * ft) for ft, off in zip(SIZES, OFFS)]
    # cf per tile: [t_seg | p_seg]
    cf = np.concatenate(
        [np.concatenate([conf["y"][:, off:off + ft],
                         conf["x"][:, off:off + ft]], axis=1)
         for ft, off in zip(SIZES, OFFS)], axis=1).astype(bf)
    bb = np.concatenate([np.concatenate([xs, ys], axis=1)
                         for xs, ys in zip(boxs["x"], boxs["y"])],
                        axis=1).astype(bf)
    return {"cf": cf, "bb": bb}


def _in_maps(x, y):
    x = np.asarray(x, dtype=np.float32)
    y = np.asarray(y, dtype=np.float32)
    maps = []
    for i in range(M):
        sl = slice(i * BS, (i + 1) * BS)
        maps.append(_pack_core(x[sl], y[sl]))
    return maps


def _combine(outs):
    """outs: list of M (o [P, T], q [1, 3*QW+3*QWB]) -> scalar fp32 loss."""
    bg = s12 = se = face = 0.0
    A = 3 * QW
    for o, q in outs:
        o = o.astype(np.float64)
        q = q.astype(np.float64)
        se += o.sum()
        face += q[0, 0:QW].sum() + q[0, A:A + QWB].sum()
        s12 += q[0, QW:2 * QW].sum() + q[0, A + QWB:A + 2 * QWB].sum()
        bg += q[0, 2 * QW:3 * QW].sum() + q[0, A + 2 * QWB:A + 3 * QWB].sum()
    scale = 1.0 + 1.0 / face
    diff_box = scale * se / (face * 4.0)
    diff_c = scale * (-s12) / face
    diff_bg = ALPHA * (-bg) / (B * N)
    return np.asarray(diff_box + diff_c + diff_bg, dtype=np.float32)


def kernel(x, y, **run_kwargs):
    nc = _get_nc()
    res = run_bass_kernel_spmd(nc, _in_maps(x, y), core_ids=list(range(M)),
                               **run_kwargs)
    out = _combine([(res.results[i]["o"], res.results[i]["q"])
                    for i in range(M)])
    if run_kwargs:
        return out, res
    return out
